# revision 7
# baseline (speedup 1.0000x reference)
"""DenseCL forward kernel for 8 Trainium2 NeuronCores.

Sharding: data-parallel over the batch (8 batches -> 8 cores, one each);
queue_g / queue_d are replicated to every core. No collectives needed.

Per-core program (batch b):
  - normalize d_q, d_k over dim (partition reduction via ones-matmul)
  - cosine = feat_k^T feat_q scaled per-key-position by 1/||feat_k col||
    (the per-query-position scale does not affect the argmax, so feat_q is
    never normalized) -- computed in TRUE fp32 on the PE (fp32r's fp22
    truncation is not precise enough: min top-2 argmax gap is ~9e-6)
  - row-wise argmax via DVE max/max_index
  - gather d_qn columns at matched indices with a one-hot matmul
    (one-hot entries are 5.0 = 1/TAU, folding the temperature in; exact
    in any dtype since each output element has exactly one nonzero term)
  - pos_d = colwise <d_kn, d_qg>, neg_d = queue_d^T @ d_qg  (fp32r)
  - pos_g/neg_g analogous for the global branch
Outputs per core: out_g [1, 1+K], out_d [1+K, S]. Host stacks them.

The queues are pre-rounded to fp22 on the host (same rounding the DVE
data converter would apply) so they can be DMA'd straight into float32r
tiles without an on-device conversion pass.
"""

import os
import sys

if "/opt/trn_rl_repo" not in sys.path:
    sys.path.insert(0, "/opt/trn_rl_repo")

import numpy as np

# bisect toggles (default = fast path)
KV_PS2 = os.environ.get("KV_PS2", "1") == "1"      # 2-bank PSUM tiles
KV_DMA3D = os.environ.get("KV_DMA3D", "1") == "1"  # rearranged 3D DMA APs

BS, DIM, S, CF, K = 8, 128, 1024, 512, 16384
TAU = 0.2
INV_TAU = 1.0 / TAU
N_CORES = 8

_CACHE = {}


def _round_fp22(a):
    """Round fp32 ndarray to fp22 (e8m13) — float32r's stored precision."""
    u = np.ascontiguousarray(a, dtype=np.float32).view(np.uint32)
    u = (u + 0x100) & np.uint32(0xFFFFFE00)
    return u.view(np.float32)


def _build_program():
    from concourse import bacc, tile, mybir

    F32 = mybir.dt.float32
    F32R = mybir.dt.float32r
    U32 = mybir.dt.uint32
    ALU = mybir.AluOpType

    nc = bacc.Bacc("TRN2", target_bir_lowering=False, debug=False,
                   num_devices=N_CORES)

    # --- DRAM I/O (per core) ---
    gq_d = nc.dram_tensor("gq", [DIM, 1], F32, kind="ExternalInput")
    gk_d = nc.dram_tensor("gk", [DIM, 1], F32, kind="ExternalInput")
    dq_d = nc.dram_tensor("dq", [DIM, S], F32, kind="ExternalInput")
    dk_d = nc.dram_tensor("dk", [DIM, S], F32, kind="ExternalInput")
    fq_d = nc.dram_tensor("fq", [CF, S], F32, kind="ExternalInput")
    fk_d = nc.dram_tensor("fk", [CF, S], F32, kind="ExternalInput")
    qg_d = nc.dram_tensor("qg", [DIM, K], F32R, kind="ExternalInput")
    qd_d = nc.dram_tensor("qd", [DIM, K], F32R, kind="ExternalInput")
    og_d = nc.dram_tensor("out_g", [1, 1 + K], F32, kind="ExternalOutput")
    od_d = nc.dram_tensor("out_d", [1 + K, S], F32, kind="ExternalOutput")

    # --- inline constants (packed into the NEFF) ---
    eye_t = nc.inline_tensor(np.eye(DIM, dtype=np.float32), name="eye128")
    ones_row_t = nc.inline_tensor(np.ones((1, DIM), dtype=np.float32),
                                  name="ones_row")
    ones_col_t = nc.inline_tensor(np.ones((DIM, 1), dtype=np.float32),
                                  name="ones_col")
    iota8_t = nc.inline_tensor(
        (np.arange(DIM, dtype=np.float32)[:, None]
         + 128.0 * np.arange(8, dtype=np.float32)[None, :]),
        name="iota8")

    NCH = CF // DIM  # 4 contraction chunks for the cosine
    NJT = S // DIM   # 8 j-tiles / transpose chunks

    with tile.TileContext(nc) as tc:
        with tc.tile_pool(name="const", bufs=1) as cpool, \
             tc.tile_pool(name="feat", bufs=1) as fpool, \
             tc.tile_pool(name="dpool", bufs=1) as dpool, \
             tc.tile_pool(name="small", bufs=1) as spool, \
             tc.tile_pool(name="cos", bufs=2) as cos_pool, \
             tc.tile_pool(name="onehot", bufs=2) as oh_pool, \
             tc.tile_pool(name="scratch", bufs=2) as scpool, \
             tc.tile_pool(name="qstream", bufs=2) as qpool, \
             tc.tile_pool(name="outp", bufs=2) as opool, \
             tc.tile_pool(name="ps", bufs=(2 if KV_PS2 else 6),
                          space="PSUM") as psp, \
             tc.tile_pool(name="ps2", bufs=(3 if KV_PS2 else 1),
                          space="PSUM") as psp2:

            cnt = [0]

            def ps_tile(shape=(1, 512)):
                # small 1-bank PSUM tiles (ssq rows, broadcasts, neg_g)
                cnt[0] += 1
                return psp.tile(list(shape), F32, tag="ps",
                                name=f"ps{cnt[0]}")

            def ps2_tile():
                # 2-bank [128, 1024] PSUM tiles (cosine, gather, neg_d)
                cnt[0] += 1
                return psp2.tile([DIM, S], F32, tag="ps2",
                                 name=f"ps2_{cnt[0]}")

            # ---- constants ----
            eye = cpool.tile([DIM, DIM], F32, tag="eye")
            ones_row = cpool.tile([1, DIM], F32, tag="ones_row")
            ones_col = cpool.tile([DIM, 1], F32, tag="ones_col")
            iota8 = cpool.tile([DIM, 8], F32, tag="iota8")
            nc.sync.dma_start(eye[:], eye_t.ap())
            nc.sync.dma_start(ones_row[:], ones_row_t.ap())
            nc.sync.dma_start(ones_col[:], ones_col_t.ap())
            nc.sync.dma_start(iota8[:], iota8_t.ap())

            # ---- input loads ----
            gq = spool.tile([DIM, 1], F32, tag="gq")
            gk = spool.tile([DIM, 1], F32, tag="gk")
            nc.sync.dma_start(gq[:], gq_d.ap())
            nc.sync.dma_start(gk[:], gk_d.ap())

            # feats as [128, 4 * S]: chunk c of 128 channels at cols c*S
            fq = fpool.tile([DIM, NCH * S], F32, tag="fq")
            fk = fpool.tile([DIM, NCH * S], F32, tag="fk")
            if KV_DMA3D:
                nc.sync.dma_start(
                    fq[:].rearrange("p (c s) -> p c s", c=NCH),
                    fq_d.ap().rearrange("(c p) s -> p c s", c=NCH))
                nc.sync.dma_start(
                    fk[:].rearrange("p (c s) -> p c s", c=NCH),
                    fk_d.ap().rearrange("(c p) s -> p c s", c=NCH))
            else:
                for c in range(NCH):
                    nc.sync.dma_start(fq[:, c * S:(c + 1) * S],
                                      fq_d.ap()[c * DIM:(c + 1) * DIM, :])
                    nc.sync.dma_start(fk[:, c * S:(c + 1) * S],
                                      fk_d.ap()[c * DIM:(c + 1) * DIM, :])

            dq = dpool.tile([DIM, S], F32, tag="dq")
            dk = dpool.tile([DIM, S], F32, tag="dk")
            nc.sync.dma_start(dq[:], dq_d.ap())
            nc.sync.dma_start(dk[:], dk_d.ap())

            # =========================================================
            # global branch scalars
            # =========================================================
            def col_sumsq(col, tag):
                sq = spool.tile([DIM, 1], F32, tag=tag + "_sq")
                nc.vector.tensor_mul(sq[:], col[:], col[:])
                ssq = spool.tile([1, 1], F32, tag=tag + "_ssq")
                nc.gpsimd.tensor_reduce(ssq[:], sq[:],
                                        axis=mybir.AxisListType.C,
                                        op=ALU.add)
                nrm = spool.tile([1, 1], F32, tag=tag + "_nrm")
                nc.scalar.sqrt(nrm[:], ssq[:])
                rsq = spool.tile([1, 1], F32, tag=tag + "_rsq")
                nc.vector.reciprocal(rsq[:], nrm[:])
                return rsq

            rsq_gq = col_sumsq(gq, "gq")
            rsq_gk = col_sumsq(gk, "gk")

            # pos_g = <gq, gk> * rsq_gq * rsq_gk / TAU  -> out_g[0, 0]
            gqk = spool.tile([DIM, 1], F32, tag="gqk")
            nc.vector.tensor_mul(gqk[:], gq[:], gk[:])
            dot_g = spool.tile([1, 1], F32, tag="dot_g")
            nc.gpsimd.tensor_reduce(dot_g[:], gqk[:],
                                    axis=mybir.AxisListType.C, op=ALU.add)
            posg = spool.tile([1, 1], F32, tag="posg")
            nc.vector.tensor_scalar(posg[:], dot_g[:], rsq_gq[:], rsq_gk[:],
                                    op0=ALU.mult, op1=ALU.mult)
            posg5 = spool.tile([1, 1], F32, tag="posg5")
            nc.scalar.mul(posg5[:], posg[:], INV_TAU)
            nc.sync.dma_start(og_d.ap()[0:1, 0:1], posg5[:])

            # gq_n5 = gq * (rsq_gq / TAU), broadcast via rank-1 matmul
            rsq5 = spool.tile([1, 1], F32, tag="rsq5")
            nc.scalar.mul(rsq5[:], rsq_gq[:], INV_TAU)
            ps_b = ps_tile((DIM, 1))
            nc.tensor.matmul(ps_b[:], ones_row[:], rsq5[:],
                             start=True, stop=True)
            gq_n5 = spool.tile([DIM, 1], F32R, tag="gq_n5")
            nc.vector.tensor_mul(gq_n5[:], gq[:], ps_b[:])

            # =========================================================
            # normalize d_q / d_k over dim (partition axis)
            # =========================================================
            def normalize_cols(x, tag):
                """x [128, S] -> x / ||x||_col, via ones-matmul colsums."""
                xsq = scpool.tile([DIM, S], F32, tag="xsq", name=tag + "_xsq")
                nc.scalar.square(xsq[:], x[:])
                ps_s = ps_tile()
                ps_s2 = ps_tile()
                nc.tensor.matmul(ps_s[:], ones_col[:], xsq[:, 0:512],
                                 start=True, stop=True)
                nc.tensor.matmul(ps_s2[:], ones_col[:], xsq[:, 512:S],
                                 start=True, stop=True)
                nrm = scpool.tile([1, S], F32, tag="nrm_row",
                                  name=tag + "_nrm")
                nc.scalar.sqrt(nrm[:, 0:512], ps_s[:])
                nc.scalar.sqrt(nrm[:, 512:S], ps_s2[:])
                rsq = scpool.tile([1, S], F32, tag="rsq_row",
                                  name=tag + "_rsq")
                nc.vector.reciprocal(rsq[:], nrm[:])
                xn = dpool.tile([DIM, S], F32, tag=tag + "_xn")
                for h in range(2):
                    ps_h = ps_tile((DIM, 512))
                    nc.tensor.matmul(ps_h[:], ones_row[:],
                                     rsq[:, h * 512:(h + 1) * 512],
                                     start=True, stop=True)
                    nc.vector.tensor_mul(xn[:, h * 512:(h + 1) * 512],
                                         x[:, h * 512:(h + 1) * 512], ps_h[:])
                return xn

            dqn = normalize_cols(dq, "dq")
            dkn = normalize_cols(dk, "dk")

            # transpose dqn -> dqnT (for the one-hot gather contraction)
            dqnT = dpool.tile([DIM, S], F32R, tag="dqnT")
            for c in range(NJT):
                ps_t = ps_tile((DIM, DIM))
                nc.tensor.transpose(ps_t[:], dqn[:, c * DIM:(c + 1) * DIM],
                                    eye[:])
                if c % 2 == 0:
                    nc.vector.tensor_copy(dqnT[:, c * DIM:(c + 1) * DIM],
                                          ps_t[:])
                else:
                    nc.scalar.copy(dqnT[:, c * DIM:(c + 1) * DIM], ps_t[:])

            # rsq of feat_k columns (feat_q scale drops out of the argmax)
            ps_f = [ps_tile() for _ in range(2)]
            for c in range(NCH):
                fsq = scpool.tile([DIM, S], F32, tag="xsq", name=f"fksq{c}")
                nc.scalar.square(fsq[:], fk[:, c * S:(c + 1) * S])
                for h in range(2):
                    nc.tensor.matmul(
                        ps_f[h][:], ones_col[:],
                        fsq[:, h * 512:(h + 1) * 512],
                        start=(c == 0), stop=(c == NCH - 1))
            fk_nrm = scpool.tile([1, S], F32, tag="nrm_row", name="fk_nrm")
            nc.scalar.sqrt(fk_nrm[:, 0:512], ps_f[0][:])
            nc.scalar.sqrt(fk_nrm[:, 512:S], ps_f[1][:])
            fk_rsq = spool.tile([1, S], F32, tag="fk_rsq")
            nc.vector.reciprocal(fk_rsq[:], fk_nrm[:])
            rsqk_bc = fpool.tile([DIM, S], F32, tag="rsqk_bc")
            for h in range(2):
                ps_h = ps_tile((DIM, 512))
                nc.tensor.matmul(ps_h[:], ones_row[:],
                                 fk_rsq[:, h * 512:(h + 1) * 512],
                                 start=True, stop=True)
                nc.vector.tensor_copy(rsqk_bc[:, h * 512:(h + 1) * 512],
                                      ps_h[:])

            # =========================================================
            # cosine (TRUE fp32) + argmax  -> m_row [1, S]
            # =========================================================
            m_row = spool.tile([1, S], F32, tag="m_row")
            for j in range(NJT):
                if KV_PS2:
                    ps_c = ps2_tile()
                    ph0, ph1 = ps_c[:, 0:512], ps_c[:, 512:S]
                else:
                    ph0 = ps_tile((DIM, 512))
                    ph1 = ps_tile((DIM, 512))
                for c in range(NCH):
                    lhs = fq[:, c * S + j * DIM: c * S + (j + 1) * DIM]
                    nc.tensor.matmul(ph0, lhs,
                                     fk[:, c * S: c * S + 512],
                                     start=(c == 0), stop=(c == NCH - 1))
                    nc.tensor.matmul(ph1, lhs,
                                     fk[:, c * S + 512: c * S + 1024],
                                     start=(c == 0), stop=(c == NCH - 1))
                cos_sb = cos_pool.tile([DIM, S], F32, tag="cos_sb")
                if KV_PS2:
                    nc.vector.tensor_mul(cos_sb[:], ps_c[:], rsqk_bc[:])
                else:
                    nc.vector.tensor_mul(cos_sb[:, 0:512], ph0,
                                         rsqk_bc[:, 0:512])
                    nc.vector.tensor_mul(cos_sb[:, 512:S], ph1,
                                         rsqk_bc[:, 512:S])
                mx = cos_pool.tile([DIM, 8], F32, tag="mx")
                mi = cos_pool.tile([DIM, 8], U32, tag="mi")
                nc.vector.max(mx[:], cos_sb[:])
                nc.vector.max_index(mi[:], mx[:], cos_sb[:])
                mif = cos_pool.tile([DIM, 1], F32, tag="mif")
                nc.vector.tensor_copy(mif[:], mi[:, 0:1])
                nc.sync.dma_start(m_row[:, j * DIM:(j + 1) * DIM], mif[:])

            # broadcast m_row over partitions
            m_bc = fpool.tile([DIM, S], F32, tag="m_bc")
            for h in range(2):
                ps_h = ps_tile((DIM, 512))
                nc.tensor.matmul(ps_h[:], ones_row[:],
                                 m_row[:, h * 512:(h + 1) * 512],
                                 start=True, stop=True)
                nc.vector.tensor_copy(m_bc[:, h * 512:(h + 1) * 512], ps_h[:])

            # one-hot gather: dqg = 5 * dqn[:, m] (fp32r exact: single
            # nonzero per column)
            if KV_PS2:
                ps_g = ps2_tile()
                pg0, pg1 = ps_g[:, 0:512], ps_g[:, 512:S]
            else:
                pg0 = ps_tile((DIM, 512))
                pg1 = ps_tile((DIM, 512))
            for c in range(NJT):
                oh = oh_pool.tile([DIM, S], F32R, tag="oh")
                nc.vector.tensor_scalar(oh[:], m_bc[:], iota8[:, c:c + 1],
                                        INV_TAU, op0=ALU.is_equal,
                                        op1=ALU.mult)
                nc.tensor.matmul(pg0,
                                 dqnT[:, c * DIM:(c + 1) * DIM],
                                 oh[:, 0:512],
                                 start=(c == 0), stop=(c == NJT - 1))
                nc.tensor.matmul(pg1,
                                 dqnT[:, c * DIM:(c + 1) * DIM],
                                 oh[:, 512:S],
                                 start=(c == 0), stop=(c == NJT - 1))
            dqg = dpool.tile([DIM, S], F32R, tag="dqg")
            if KV_PS2:
                nc.vector.tensor_copy(dqg[:], ps_g[:])
            else:
                nc.vector.tensor_copy(dqg[:, 0:512], pg0)
                nc.vector.tensor_copy(dqg[:, 512:S], pg1)

            # =========================================================
            # pos_d -> out_d row 0
            # =========================================================
            pd = dpool.tile([DIM, S], F32, tag="pd")
            nc.vector.tensor_mul(pd[:], dkn[:], dqg[:].bitcast(F32))
            posd_row = spool.tile([1, S], F32, tag="posd_row")
            for h in range(2):
                ps_h = ps_tile()
                nc.tensor.matmul(ps_h[:], ones_col[:],
                                 pd[:, h * 512:(h + 1) * 512],
                                 start=True, stop=True)
                nc.vector.tensor_copy(posd_row[:, h * 512:(h + 1) * 512],
                                      ps_h[:])
            nc.sync.dma_start(od_d.ap()[0:1, :], posd_row[:])

            # =========================================================
            # neg_d: queue_d^T @ dqg  (the bulk: 128 q-tiles x [128, 1024])
            # =========================================================
            NQC = 8   # queue chunks of 2048 columns (1 MB loads)
            GRP = 4   # q-tiles (of 128) per output tile / DMA (2 MB stores)
            for t in range(NQC):
                qd_ch = qpool.tile([DIM, 2048], F32R, tag="qd_ch")
                nc.sync.dma_start(qd_ch[:],
                                  qd_d.ap()[:, t * 2048:(t + 1) * 2048])
                for g in range(2048 // (GRP * DIM)):  # 4 groups of 4 q-tiles
                    ot = opool.tile([DIM, GRP * S], F32, tag="ot")
                    q0 = t * 2048 + g * GRP * DIM
                    for u in range(GRP):
                        lhs = qd_ch[:, (g * GRP + u) * DIM:
                                    (g * GRP + u + 1) * DIM]
                        if KV_PS2:
                            ps_n = ps2_tile()
                            pn0, pn1 = ps_n[:, 0:512], ps_n[:, 512:S]
                        else:
                            pn0 = ps_tile((DIM, 512))
                            pn1 = ps_tile((DIM, 512))
                        nc.tensor.matmul(pn0, lhs, dqg[:, 0:512],
                                         start=True, stop=True)
                        nc.tensor.matmul(pn1, lhs, dqg[:, 512:S],
                                         start=True, stop=True)
                        dve_turn = (t * 16 + g * GRP + u) % 8 < 5
                        if KV_PS2:
                            if dve_turn:
                                nc.vector.tensor_copy(
                                    ot[:, u * S:(u + 1) * S], ps_n[:])
                            else:
                                nc.scalar.copy(
                                    ot[:, u * S:(u + 1) * S], ps_n[:])
                        else:
                            eng = nc.vector.tensor_copy if dve_turn \
                                else nc.scalar.copy
                            eng(ot[:, u * S: u * S + 512], pn0)
                            eng(ot[:, u * S + 512:(u + 1) * S], pn1)
                    if KV_DMA3D:
                        nc.sync.dma_start(
                            od_d.ap()[1 + q0: 1 + q0 + GRP * DIM, :]
                            .rearrange("(b p) s -> p b s", b=GRP),
                            ot[:].rearrange("p (b s) -> p b s", b=GRP))
                    else:
                        for u in range(GRP):
                            nc.sync.dma_start(
                                od_d.ap()[1 + q0 + u * DIM:
                                          1 + q0 + (u + 1) * DIM, :],
                                ot[:, u * S:(u + 1) * S])

            # =========================================================
            # neg_g: gq_n5^T @ queue_g -> out_g[0, 1:]
            # =========================================================
            for t in range(NQC):
                qg_ch = qpool.tile([DIM, 2048], F32R, tag="qg_ch")
                nc.sync.dma_start(qg_ch[:],
                                  qg_d.ap()[:, t * 2048:(t + 1) * 2048])
                ng_sb = qpool.tile([1, 2048], F32, tag="ng_sb")
                for v in range(4):
                    ps_n = ps_tile()
                    nc.tensor.matmul(ps_n[:], gq_n5[:],
                                     qg_ch[:, v * 512:(v + 1) * 512],
                                     start=True, stop=True)
                    if v % 2 == 0:
                        nc.vector.tensor_copy(
                            ng_sb[:, v * 512:(v + 1) * 512], ps_n[:])
                    else:
                        nc.scalar.copy(
                            ng_sb[:, v * 512:(v + 1) * 512], ps_n[:])
                nc.sync.dma_start(
                    og_d.ap()[0:1, 1 + t * 2048: 1 + (t + 1) * 2048],
                    ng_sb[:])

    nc.compile()
    return nc


def _get_program():
    if "nc" not in _CACHE:
        _CACHE["nc"] = _build_program()
    return _CACHE["nc"]


def _in_maps(inputs):
    qg = _round_fp22(inputs["queue_g"])
    qd = _round_fp22(inputs["queue_d"])
    maps = []
    for b in range(N_CORES):
        maps.append({
            "gq": np.ascontiguousarray(
                inputs["g_q"][b].reshape(DIM, 1), dtype=np.float32),
            "gk": np.ascontiguousarray(
                inputs["g_k"][b].reshape(DIM, 1), dtype=np.float32),
            "dq": np.ascontiguousarray(inputs["d_q"][b], dtype=np.float32),
            "dk": np.ascontiguousarray(inputs["d_k"][b], dtype=np.float32),
            "fq": np.ascontiguousarray(inputs["feat_q"][b], dtype=np.float32),
            "fk": np.ascontiguousarray(inputs["feat_k"][b], dtype=np.float32),
            "qg": qg,
            "qd": qd,
        })
    return maps


def run_sharded(inputs, trace=False):
    """Run the SPMD kernel; returns (out_g [8,1+K], out_d [8,1+K,S], results)."""
    from concourse.bass_utils import run_bass_kernel_spmd

    nc = _get_program()
    res = run_bass_kernel_spmd(nc, _in_maps(inputs), list(range(N_CORES)),
                               trace=trace)
    out_g = np.stack([res.results[b]["out_g"][0] for b in range(N_CORES)])
    out_d = np.stack([res.results[b]["out_d"] for b in range(N_CORES)])
    return out_g, out_d, res


def benchmark_hw(inputs, iters=16):
    """Median per-execution time with device-resident inputs.

    Mirrors run_bass_via_pjrt's multi-core path, but keeps inputs on device
    and ping-pongs the donated output buffers, so repeated calls measure
    dispatch + execution only. Returns (per_exec_seconds, outputs).
    """
    import jax
    import time
    from jax.sharding import Mesh, NamedSharding, PartitionSpec
    from jax.experimental.shard_map import shard_map
    from concourse import bass2jax, mybir
    from concourse.bass2jax import _bass_exec_p, partition_id_tensor

    bass2jax.install_neuronx_cc_hook()
    nc = _get_program()
    part_name = nc.partition_id_tensor.name if nc.partition_id_tensor else None

    in_names, out_names, out_avals, zero_shapes = [], [], [], []
    for alloc in nc.m.functions[0].allocations:
        if not isinstance(alloc, mybir.MemoryLocationSet):
            continue
        name = alloc.memorylocations[0].name
        if alloc.kind == "ExternalInput":
            if name != part_name:
                in_names.append(name)
        elif alloc.kind == "ExternalOutput":
            shape = list(alloc.tensor_shape)
            out_names.append(name)
            out_avals.append(
                jax.core.ShapedArray(shape, mybir.dt.np(alloc.dtype)))
            zero_shapes.append((shape, mybir.dt.np(alloc.dtype)))
    n_params, n_outs = len(in_names), len(out_names)
    all_names = list(in_names) + list(out_names)
    if part_name is not None:
        all_names.append(part_name)

    def _body(*args):
        operands = list(args)
        if part_name is not None:
            operands.append(partition_id_tensor())
        return tuple(_bass_exec_p.bind(
            *operands,
            out_avals=tuple(out_avals),
            in_names=tuple(all_names),
            out_names=tuple(out_names),
            lowering_input_output_aliases=(),
            sim_require_finite=True,
            sim_require_nnan=True,
            nc=nc,
        ))

    devices = jax.devices()[:N_CORES]
    mesh = Mesh(np.asarray(devices), ("core",))
    sharded = jax.jit(
        shard_map(_body, mesh=mesh,
                  in_specs=(PartitionSpec("core"),) * (n_params + n_outs),
                  out_specs=(PartitionSpec("core"),) * n_outs,
                  check_rep=False),
        donate_argnums=tuple(range(n_params, n_params + n_outs)),
        keep_unused=True)

    maps = _in_maps(inputs)
    per_core = [[m[name] for name in in_names] for m in maps]
    shard = NamedSharding(mesh, PartitionSpec("core"))
    d_ins = [
        jax.device_put(
            np.concatenate([per_core[c][i] for c in range(N_CORES)], axis=0),
            shard)
        for i in range(n_params)
    ]
    outs = tuple(
        jax.device_put(np.zeros((N_CORES * s[0], *s[1:]), dt), shard)
        for s, dt in zero_shapes)

    outs = sharded(*d_ins, *outs)   # compile + first run
    jax.block_until_ready(outs)
    res0 = {name: np.asarray(outs[i]).reshape(N_CORES, *out_avals[i].shape)
            for i, name in enumerate(out_names)}

    def timed(k):
        nonlocal outs
        t0 = time.perf_counter()
        for _ in range(k):
            outs = sharded(*d_ins, *outs)
        jax.block_until_ready(outs)
        return time.perf_counter() - t0

    timed(2)  # warm
    t1 = min(timed(1) for _ in range(3))
    tk = min(timed(iters) for _ in range(3))
    per_exec = (tk - t1) / (iters - 1)
    return per_exec, res0


def kernel(g_q, g_k, d_q, d_k, feat_q, feat_k, queue_g, queue_d):
    inputs = dict(g_q=g_q, g_k=g_k, d_q=d_q, d_k=d_k, feat_q=feat_q,
                  feat_k=feat_k, queue_g=queue_g, queue_d=queue_d)
    inputs = {k: np.asarray(v) for k, v in inputs.items()}
    out_g, out_d, _ = run_sharded(inputs, trace=False)
    target_g = np.zeros((BS,), dtype=np.int32)
    target_d = np.zeros((BS, S), dtype=np.int32)
    return out_g, target_g, out_d, target_d


# revision 12
# speedup vs baseline: 6.2392x; 6.2392x over previous
"""DenseCL forward kernel for 8 Trainium2 NeuronCores.

Sharding: data-parallel over the batch (8 batches -> 8 cores, one each);
queue_g / queue_d are replicated to every core. No collectives needed.

Per-core program (batch b):
  - normalize d_q, d_k over dim (partition reduction via ones-matmul)
  - cosine = feat_k^T feat_q scaled per-key-position by 1/||feat_k col||
    (the per-query-position scale does not affect the argmax, so feat_q is
    never normalized) -- computed in TRUE fp32 on the PE (fp32r's fp22
    truncation is not precise enough: min top-2 argmax gap is ~9e-6)
  - row-wise argmax via DVE max/max_index
  - gather d_qn columns at matched indices with a one-hot matmul
    (one-hot entries are 5.0 = 1/TAU, folding the temperature in; exact
    in any dtype since each output element has exactly one nonzero term)
  - pos_d = colwise <d_kn, d_qg>, neg_d = queue_d^T @ d_qg  (fp32r)
  - pos_g/neg_g analogous for the global branch
Outputs per core: out_g [1, 1+K], out_d [1+K, S]. Host stacks them.

The queues are pre-rounded to fp22 on the host (same rounding the DVE
data converter would apply) so they can be DMA'd straight into float32r
tiles without an on-device conversion pass.
"""

import os
import sys

if "/opt/trn_rl_repo" not in sys.path:
    sys.path.insert(0, "/opt/trn_rl_repo")

import numpy as np

# bisect toggles (default = fast path)
KV_PS2 = os.environ.get("KV_PS2", "0") == "1"      # 2-bank PSUM tiles (FATAL: psum bank collision)
KV_DMA3D = os.environ.get("KV_DMA3D", "1") == "1"  # rearranged 3D DMA APs

BS, DIM, S, CF, K = 8, 128, 1024, 512, 16384
TAU = 0.2
INV_TAU = 1.0 / TAU
N_CORES = 8

_CACHE = {}


def _round_fp22(a):
    """Round fp32 ndarray to fp22 (e8m13) — float32r's stored precision."""
    u = np.ascontiguousarray(a, dtype=np.float32).view(np.uint32)
    u = (u + 0x100) & np.uint32(0xFFFFFE00)
    return u.view(np.float32)


def _build_program():
    from concourse import bacc, tile, mybir

    F32 = mybir.dt.float32
    F32R = mybir.dt.float32r
    U32 = mybir.dt.uint32
    ALU = mybir.AluOpType

    nc = bacc.Bacc("TRN2", target_bir_lowering=False, debug=False,
                   num_devices=N_CORES)

    # --- DRAM I/O (per core) ---
    gq_d = nc.dram_tensor("gq", [DIM, 1], F32, kind="ExternalInput")
    gk_d = nc.dram_tensor("gk", [DIM, 1], F32, kind="ExternalInput")
    dq_d = nc.dram_tensor("dq", [DIM, S], F32, kind="ExternalInput")
    dk_d = nc.dram_tensor("dk", [DIM, S], F32, kind="ExternalInput")
    fq_d = nc.dram_tensor("fq", [CF, S], F32, kind="ExternalInput")
    fk_d = nc.dram_tensor("fk", [CF, S], F32, kind="ExternalInput")
    qg_d = nc.dram_tensor("qg", [DIM, K], F32R, kind="ExternalInput")
    qd_d = nc.dram_tensor("qd", [DIM, K], F32R, kind="ExternalInput")
    og_d = nc.dram_tensor("out_g", [1, 1 + K], F32, kind="ExternalOutput")
    od_d = nc.dram_tensor("out_d", [1 + K, S], F32, kind="ExternalOutput")

    # --- inline constants (packed into the NEFF) ---
    eye_t = nc.inline_tensor(np.eye(DIM, dtype=np.float32), name="eye128")
    ones_row_t = nc.inline_tensor(np.ones((1, DIM), dtype=np.float32),
                                  name="ones_row")
    ones_col_t = nc.inline_tensor(np.ones((DIM, 1), dtype=np.float32),
                                  name="ones_col")
    iota8_t = nc.inline_tensor(
        (np.arange(DIM, dtype=np.float32)[:, None]
         + 128.0 * np.arange(8, dtype=np.float32)[None, :]),
        name="iota8")

    NCH = CF // DIM  # 4 contraction chunks for the cosine
    NJT = S // DIM   # 8 j-tiles / transpose chunks
    NQC = 8          # queue chunks of 2048 columns (1 MB loads)

    with tile.TileContext(nc) as tc:
        with tc.tile_pool(name="const", bufs=1) as cpool, \
             tc.tile_pool(name="feat", bufs=1) as fpool, \
             tc.tile_pool(name="dpool", bufs=1) as dpool, \
             tc.tile_pool(name="small", bufs=1) as spool, \
             tc.tile_pool(name="cos", bufs=2) as cos_pool, \
             tc.tile_pool(name="onehot", bufs=2) as oh_pool, \
             tc.tile_pool(name="scratch", bufs=2) as scpool, \
             tc.tile_pool(name="qstream", bufs=3) as qpool, \
             tc.tile_pool(name="qgstream", bufs=2) as qgpool, \
             tc.tile_pool(name="outp", bufs=2) as opool, \
             tc.tile_pool(name="ps", bufs=(2 if KV_PS2 else 6),
                          space="PSUM") as psp, \
             tc.tile_pool(name="ps2", bufs=(3 if KV_PS2 else 1),
                          space="PSUM") as psp2:

            cnt = [0]

            def ps_tile(shape=(1, 512)):
                # small 1-bank PSUM tiles (ssq rows, broadcasts, neg_g)
                cnt[0] += 1
                return psp.tile(list(shape), F32, tag="ps",
                                name=f"ps{cnt[0]}")

            def ps2_tile():
                # 2-bank [128, 1024] PSUM tiles (cosine, gather, neg_d)
                cnt[0] += 1
                return psp2.tile([DIM, S], F32, tag="ps2",
                                 name=f"ps2_{cnt[0]}")

            # ---- constants ----
            eye = cpool.tile([DIM, DIM], F32, tag="eye")
            ones_row = cpool.tile([1, DIM], F32, tag="ones_row")
            ones_col = cpool.tile([DIM, 1], F32, tag="ones_col")
            iota8 = cpool.tile([DIM, 8], F32, tag="iota8")
            nc.sync.dma_start(eye[:], eye_t.ap())
            nc.sync.dma_start(ones_row[:], ones_row_t.ap())
            nc.sync.dma_start(ones_col[:], ones_col_t.ap())
            nc.sync.dma_start(iota8[:], iota8_t.ap())

            # ---- input loads ----
            gq = spool.tile([DIM, 1], F32, tag="gq")
            gk = spool.tile([DIM, 1], F32, tag="gk")
            nc.sync.dma_start(gq[:], gq_d.ap())
            nc.sync.dma_start(gk[:], gk_d.ap())

            # feats as [128, 4 * S]: chunk c of 128 channels at cols c*S
            fq = fpool.tile([DIM, NCH * S], F32, tag="fq")
            fk = fpool.tile([DIM, NCH * S], F32, tag="fk")
            if KV_DMA3D:
                nc.sync.dma_start(
                    fq[:].rearrange("p (c s) -> p c s", c=NCH),
                    fq_d.ap().rearrange("(c p) s -> p c s", c=NCH))
                nc.sync.dma_start(
                    fk[:].rearrange("p (c s) -> p c s", c=NCH),
                    fk_d.ap().rearrange("(c p) s -> p c s", c=NCH))
            else:
                for c in range(NCH):
                    nc.sync.dma_start(fq[:, c * S:(c + 1) * S],
                                      fq_d.ap()[c * DIM:(c + 1) * DIM, :])
                    nc.sync.dma_start(fk[:, c * S:(c + 1) * S],
                                      fk_d.ap()[c * DIM:(c + 1) * DIM, :])

            dq = dpool.tile([DIM, S], F32, tag="dq")
            dk = dpool.tile([DIM, S], F32, tag="dk")
            nc.sync.dma_start(dq[:], dq_d.ap())
            nc.sync.dma_start(dk[:], dk_d.ap())

            # =========================================================
            # global branch scalars
            # =========================================================
            def col_sumsq(col, tag):
                sq = spool.tile([DIM, 1], F32, tag=tag + "_sq")
                nc.vector.tensor_mul(sq[:], col[:], col[:])
                ssq = spool.tile([1, 1], F32, tag=tag + "_ssq")
                nc.gpsimd.tensor_reduce(ssq[:], sq[:],
                                        axis=mybir.AxisListType.C,
                                        op=ALU.add)
                nrm = spool.tile([1, 1], F32, tag=tag + "_nrm")
                nc.scalar.sqrt(nrm[:], ssq[:])
                rsq = spool.tile([1, 1], F32, tag=tag + "_rsq")
                nc.vector.reciprocal(rsq[:], nrm[:])
                return rsq

            rsq_gq = col_sumsq(gq, "gq")
            rsq_gk = col_sumsq(gk, "gk")

            # pos_g = <gq, gk> * rsq_gq * rsq_gk / TAU  -> out_g[0, 0]
            gqk = spool.tile([DIM, 1], F32, tag="gqk")
            nc.vector.tensor_mul(gqk[:], gq[:], gk[:])
            dot_g = spool.tile([1, 1], F32, tag="dot_g")
            nc.gpsimd.tensor_reduce(dot_g[:], gqk[:],
                                    axis=mybir.AxisListType.C, op=ALU.add)
            posg = spool.tile([1, 1], F32, tag="posg")
            nc.vector.tensor_scalar(posg[:], dot_g[:], rsq_gq[:], rsq_gk[:],
                                    op0=ALU.mult, op1=ALU.mult)
            posg5 = spool.tile([1, 1], F32, tag="posg5")
            nc.scalar.mul(posg5[:], posg[:], INV_TAU)
            nc.sync.dma_start(og_d.ap()[0:1, 0:1], posg5[:])

            # gq_n5 = gq * (rsq_gq / TAU), broadcast via rank-1 matmul
            rsq5 = spool.tile([1, 1], F32, tag="rsq5")
            nc.scalar.mul(rsq5[:], rsq_gq[:], INV_TAU)
            ps_b = ps_tile((DIM, 1))
            nc.tensor.matmul(ps_b[:], ones_row[:], rsq5[:],
                             start=True, stop=True)
            gq_n5 = spool.tile([DIM, 1], F32R, tag="gq_n5")
            nc.vector.tensor_mul(gq_n5[:], gq[:], ps_b[:])

            # =========================================================
            # neg_g: gq_n5^T @ queue_g -> out_g[0, 1:]
            # =========================================================
            for t in range(NQC):
                qg_ch = qgpool.tile([DIM, 2048], F32R, tag="qg_ch")
                nc.sync.dma_start(qg_ch[:],
                                  qg_d.ap()[:, t * 2048:(t + 1) * 2048])
                ng_sb = qgpool.tile([1, 2048], F32, tag="ng_sb")
                for v in range(4):
                    ps_n = ps_tile()
                    nc.tensor.matmul(ps_n[:], gq_n5[:],
                                     qg_ch[:, v * 512:(v + 1) * 512],
                                     start=True, stop=True)
                    if v % 2 == 0:
                        nc.vector.tensor_copy(
                            ng_sb[:, v * 512:(v + 1) * 512], ps_n[:])
                    else:
                        nc.scalar.copy(
                            ng_sb[:, v * 512:(v + 1) * 512], ps_n[:])
                nc.sync.dma_start(
                    og_d.ap()[0:1, 1 + t * 2048: 1 + (t + 1) * 2048],
                    ng_sb[:])


            # =========================================================
            # normalize d_q / d_k over dim (partition axis)
            # =========================================================
            def normalize_cols(x, tag):
                """x [128, S] -> x / ||x||_col, via ones-matmul colsums."""
                xsq = scpool.tile([DIM, S], F32, tag="xsq", name=tag + "_xsq")
                nc.scalar.square(xsq[:], x[:])
                ps_s = ps_tile()
                ps_s2 = ps_tile()
                nc.tensor.matmul(ps_s[:], ones_col[:], xsq[:, 0:512],
                                 start=True, stop=True)
                nc.tensor.matmul(ps_s2[:], ones_col[:], xsq[:, 512:S],
                                 start=True, stop=True)
                nrm = scpool.tile([1, S], F32, tag="nrm_row",
                                  name=tag + "_nrm")
                nc.scalar.sqrt(nrm[:, 0:512], ps_s[:])
                nc.scalar.sqrt(nrm[:, 512:S], ps_s2[:])
                rsq = scpool.tile([1, S], F32, tag="rsq_row",
                                  name=tag + "_rsq")
                nc.vector.reciprocal(rsq[:], nrm[:])
                xn = dpool.tile([DIM, S], F32, tag=tag + "_xn")
                for h in range(2):
                    ps_h = ps_tile((DIM, 512))
                    nc.tensor.matmul(ps_h[:], ones_row[:],
                                     rsq[:, h * 512:(h + 1) * 512],
                                     start=True, stop=True)
                    nc.vector.tensor_mul(xn[:, h * 512:(h + 1) * 512],
                                         x[:, h * 512:(h + 1) * 512], ps_h[:])
                return xn

            dqn = normalize_cols(dq, "dq")
            dkn = normalize_cols(dk, "dk")

            # transpose dqn -> dqnT (for the one-hot gather contraction)
            dqnT = dpool.tile([DIM, S], F32R, tag="dqnT")
            for c in range(NJT):
                ps_t = ps_tile((DIM, DIM))
                nc.tensor.transpose(ps_t[:], dqn[:, c * DIM:(c + 1) * DIM],
                                    eye[:])
                if c % 2 == 0:
                    nc.vector.tensor_copy(dqnT[:, c * DIM:(c + 1) * DIM],
                                          ps_t[:])
                else:
                    nc.scalar.copy(dqnT[:, c * DIM:(c + 1) * DIM], ps_t[:])

            # rsq of feat_k columns (feat_q scale drops out of the argmax)
            ps_f = [ps_tile() for _ in range(2)]
            for c in range(NCH):
                fsq = scpool.tile([DIM, S], F32, tag="xsq", name=f"fksq{c}")
                nc.scalar.square(fsq[:], fk[:, c * S:(c + 1) * S])
                for h in range(2):
                    nc.tensor.matmul(
                        ps_f[h][:], ones_col[:],
                        fsq[:, h * 512:(h + 1) * 512],
                        start=(c == 0), stop=(c == NCH - 1))
            fk_nrm = scpool.tile([1, S], F32, tag="nrm_row", name="fk_nrm")
            nc.scalar.sqrt(fk_nrm[:, 0:512], ps_f[0][:])
            nc.scalar.sqrt(fk_nrm[:, 512:S], ps_f[1][:])
            fk_rsq = spool.tile([1, S], F32, tag="fk_rsq")
            nc.vector.reciprocal(fk_rsq[:], fk_nrm[:])
            rsqk_bc = fpool.tile([DIM, S], F32, tag="rsqk_bc")
            for h in range(2):
                ps_h = ps_tile((DIM, 512))
                nc.tensor.matmul(ps_h[:], ones_row[:],
                                 fk_rsq[:, h * 512:(h + 1) * 512],
                                 start=True, stop=True)
                nc.vector.tensor_copy(rsqk_bc[:, h * 512:(h + 1) * 512],
                                      ps_h[:])

            # =========================================================
            # cosine (TRUE fp32) + argmax  -> m_row [1, S]
            # =========================================================
            m_row = spool.tile([1, S], F32, tag="m_row")
            for j in range(NJT):
                if KV_PS2:
                    ps_c = ps2_tile()
                    ph0, ph1 = ps_c[:, 0:512], ps_c[:, 512:S]
                else:
                    ph0 = ps_tile((DIM, 512))
                    ph1 = ps_tile((DIM, 512))
                for c in range(NCH):
                    lhs = fq[:, c * S + j * DIM: c * S + (j + 1) * DIM]
                    nc.tensor.matmul(ph0, lhs,
                                     fk[:, c * S: c * S + 512],
                                     start=(c == 0), stop=(c == NCH - 1))
                    nc.tensor.matmul(ph1, lhs,
                                     fk[:, c * S + 512: c * S + 1024],
                                     start=(c == 0), stop=(c == NCH - 1))
                cos_sb = cos_pool.tile([DIM, S], F32, tag="cos_sb")
                if KV_PS2:
                    nc.vector.tensor_mul(cos_sb[:], ps_c[:], rsqk_bc[:])
                else:
                    nc.vector.tensor_mul(cos_sb[:, 0:512], ph0,
                                         rsqk_bc[:, 0:512])
                    nc.vector.tensor_mul(cos_sb[:, 512:S], ph1,
                                         rsqk_bc[:, 512:S])
                mx = cos_pool.tile([DIM, 8], F32, tag="mx")
                mi = cos_pool.tile([DIM, 8], U32, tag="mi")
                nc.vector.max(mx[:], cos_sb[:])
                nc.vector.max_index(mi[:], mx[:], cos_sb[:])
                mif = cos_pool.tile([DIM, 1], F32, tag="mif")
                nc.vector.tensor_copy(mif[:], mi[:, 0:1])
                nc.sync.dma_start(m_row[:, j * DIM:(j + 1) * DIM], mif[:])

            # broadcast m_row over partitions
            m_bc = fpool.tile([DIM, S], F32, tag="m_bc")
            for h in range(2):
                ps_h = ps_tile((DIM, 512))
                nc.tensor.matmul(ps_h[:], ones_row[:],
                                 m_row[:, h * 512:(h + 1) * 512],
                                 start=True, stop=True)
                nc.vector.tensor_copy(m_bc[:, h * 512:(h + 1) * 512], ps_h[:])

            # one-hot gather: dqg = 5 * dqn[:, m] (fp32r exact: single
            # nonzero per column)
            if KV_PS2:
                ps_g = ps2_tile()
                pg0, pg1 = ps_g[:, 0:512], ps_g[:, 512:S]
            else:
                pg0 = ps_tile((DIM, 512))
                pg1 = ps_tile((DIM, 512))
            for c in range(NJT):
                oh = oh_pool.tile([DIM, S], F32R, tag="oh")
                nc.vector.tensor_scalar(oh[:], m_bc[:], iota8[:, c:c + 1],
                                        INV_TAU, op0=ALU.is_equal,
                                        op1=ALU.mult)
                nc.tensor.matmul(pg0,
                                 dqnT[:, c * DIM:(c + 1) * DIM],
                                 oh[:, 0:512],
                                 start=(c == 0), stop=(c == NJT - 1))
                nc.tensor.matmul(pg1,
                                 dqnT[:, c * DIM:(c + 1) * DIM],
                                 oh[:, 512:S],
                                 start=(c == 0), stop=(c == NJT - 1))
            dqg = dpool.tile([DIM, S], F32R, tag="dqg")
            if KV_PS2:
                nc.vector.tensor_copy(dqg[:], ps_g[:])
            else:
                nc.vector.tensor_copy(dqg[:, 0:512], pg0)
                nc.vector.tensor_copy(dqg[:, 512:S], pg1)

            # =========================================================
            # pos_d -> out_d row 0
            # =========================================================
            pd = scpool.tile([DIM, S], F32, tag="xsq", name="pd")
            nc.vector.tensor_mul(pd[:], dkn[:], dqg[:].bitcast(F32))
            posd_row = spool.tile([1, S], F32, tag="posd_row")
            for h in range(2):
                ps_h = ps_tile()
                nc.tensor.matmul(ps_h[:], ones_col[:],
                                 pd[:, h * 512:(h + 1) * 512],
                                 start=True, stop=True)
                nc.vector.tensor_copy(posd_row[:, h * 512:(h + 1) * 512],
                                      ps_h[:])
            nc.sync.dma_start(od_d.ap()[0:1, :], posd_row[:])

            # =========================================================
            # neg_d: queue_d^T @ dqg  (the bulk: 128 q-tiles x [128, 1024])
            # =========================================================
            GRP = 4   # q-tiles (of 128) per output tile / DMA (2 MB stores)
            for t in range(NQC):
                qd_ch = qpool.tile([DIM, 2048], F32R, tag="qd_ch")
                nc.sync.dma_start(qd_ch[:],
                                  qd_d.ap()[:, t * 2048:(t + 1) * 2048])
                for g in range(2048 // (GRP * DIM)):  # 4 groups of 4 q-tiles
                    ot = opool.tile([DIM, GRP * S], F32, tag="ot")
                    q0 = t * 2048 + g * GRP * DIM
                    for u in range(GRP):
                        lhs = qd_ch[:, (g * GRP + u) * DIM:
                                    (g * GRP + u + 1) * DIM]
                        if KV_PS2:
                            ps_n = ps2_tile()
                            pn0, pn1 = ps_n[:, 0:512], ps_n[:, 512:S]
                        else:
                            pn0 = ps_tile((DIM, 512))
                            pn1 = ps_tile((DIM, 512))
                        nc.tensor.matmul(pn0, lhs, dqg[:, 0:512],
                                         start=True, stop=True)
                        nc.tensor.matmul(pn1, lhs, dqg[:, 512:S],
                                         start=True, stop=True)
                        dve_turn = (t * 16 + g * GRP + u) % 8 < 5
                        if KV_PS2:
                            if dve_turn:
                                nc.vector.tensor_copy(
                                    ot[:, u * S:(u + 1) * S], ps_n[:])
                            else:
                                nc.scalar.copy(
                                    ot[:, u * S:(u + 1) * S], ps_n[:])
                        else:
                            eng = nc.vector.tensor_copy if dve_turn \
                                else nc.scalar.copy
                            eng(ot[:, u * S: u * S + 512], pn0)
                            eng(ot[:, u * S + 512:(u + 1) * S], pn1)
                    if KV_DMA3D:
                        nc.sync.dma_start(
                            od_d.ap()[1 + q0: 1 + q0 + GRP * DIM, :]
                            .rearrange("(b p) s -> p b s", b=GRP),
                            ot[:].rearrange("p (b s) -> p b s", b=GRP))
                    else:
                        for u in range(GRP):
                            nc.sync.dma_start(
                                od_d.ap()[1 + q0 + u * DIM:
                                          1 + q0 + (u + 1) * DIM, :],
                                ot[:, u * S:(u + 1) * S])


    nc.compile()
    return nc


def _get_program():
    if "nc" not in _CACHE:
        _CACHE["nc"] = _build_program()
    return _CACHE["nc"]


def _in_maps(inputs):
    qg = _round_fp22(inputs["queue_g"])
    qd = _round_fp22(inputs["queue_d"])
    maps = []
    for b in range(N_CORES):
        maps.append({
            "gq": np.ascontiguousarray(
                inputs["g_q"][b].reshape(DIM, 1), dtype=np.float32),
            "gk": np.ascontiguousarray(
                inputs["g_k"][b].reshape(DIM, 1), dtype=np.float32),
            "dq": np.ascontiguousarray(inputs["d_q"][b], dtype=np.float32),
            "dk": np.ascontiguousarray(inputs["d_k"][b], dtype=np.float32),
            "fq": np.ascontiguousarray(inputs["feat_q"][b], dtype=np.float32),
            "fk": np.ascontiguousarray(inputs["feat_k"][b], dtype=np.float32),
            "qg": qg,
            "qd": qd,
        })
    return maps


def run_sharded(inputs, trace=False):
    """Run the SPMD kernel; returns (out_g [8,1+K], out_d [8,1+K,S], results)."""
    from concourse.bass_utils import run_bass_kernel_spmd

    nc = _get_program()
    res = run_bass_kernel_spmd(nc, _in_maps(inputs), list(range(N_CORES)),
                               trace=trace)
    out_g = np.stack([res.results[b]["out_g"][0] for b in range(N_CORES)])
    out_d = np.stack([res.results[b]["out_d"] for b in range(N_CORES)])
    return out_g, out_d, res


def benchmark_hw(inputs, iters=16):
    """Median per-execution time with device-resident inputs.

    Mirrors run_bass_via_pjrt's multi-core path, but keeps inputs on device
    and ping-pongs the donated output buffers, so repeated calls measure
    dispatch + execution only. Returns (per_exec_seconds, outputs).
    """
    import jax
    import time
    from jax.sharding import Mesh, NamedSharding, PartitionSpec
    from jax.experimental.shard_map import shard_map
    from concourse import bass2jax, mybir
    from concourse.bass2jax import _bass_exec_p, partition_id_tensor

    bass2jax.install_neuronx_cc_hook()
    nc = _get_program()
    part_name = nc.partition_id_tensor.name if nc.partition_id_tensor else None

    in_names, out_names, out_avals, zero_shapes = [], [], [], []
    for alloc in nc.m.functions[0].allocations:
        if not isinstance(alloc, mybir.MemoryLocationSet):
            continue
        name = alloc.memorylocations[0].name
        if alloc.kind == "ExternalInput":
            if name != part_name:
                in_names.append(name)
        elif alloc.kind == "ExternalOutput":
            shape = list(alloc.tensor_shape)
            out_names.append(name)
            out_avals.append(
                jax.core.ShapedArray(shape, mybir.dt.np(alloc.dtype)))
            zero_shapes.append((shape, mybir.dt.np(alloc.dtype)))
    n_params, n_outs = len(in_names), len(out_names)
    all_names = list(in_names) + list(out_names)
    if part_name is not None:
        all_names.append(part_name)

    def _body(*args):
        operands = list(args)
        if part_name is not None:
            operands.append(partition_id_tensor())
        return tuple(_bass_exec_p.bind(
            *operands,
            out_avals=tuple(out_avals),
            in_names=tuple(all_names),
            out_names=tuple(out_names),
            lowering_input_output_aliases=(),
            sim_require_finite=True,
            sim_require_nnan=True,
            nc=nc,
        ))

    devices = jax.devices()[:N_CORES]
    mesh = Mesh(np.asarray(devices), ("core",))
    sharded = jax.jit(
        shard_map(_body, mesh=mesh,
                  in_specs=(PartitionSpec("core"),) * (n_params + n_outs),
                  out_specs=(PartitionSpec("core"),) * n_outs,
                  check_rep=False),
        donate_argnums=tuple(range(n_params, n_params + n_outs)),
        keep_unused=True)

    maps = _in_maps(inputs)
    per_core = [[m[name] for name in in_names] for m in maps]
    shard = NamedSharding(mesh, PartitionSpec("core"))
    d_ins = [
        jax.device_put(
            np.concatenate([per_core[c][i] for c in range(N_CORES)], axis=0),
            shard)
        for i in range(n_params)
    ]
    outs = tuple(
        jax.device_put(np.zeros((N_CORES * s[0], *s[1:]), dt), shard)
        for s, dt in zero_shapes)

    outs = sharded(*d_ins, *outs)   # compile + first run
    jax.block_until_ready(outs)
    res0 = {name: np.asarray(outs[i]).reshape(N_CORES, *out_avals[i].shape)
            for i, name in enumerate(out_names)}

    def timed(k):
        nonlocal outs
        t0 = time.perf_counter()
        for _ in range(k):
            outs = sharded(*d_ins, *outs)
        jax.block_until_ready(outs)
        return time.perf_counter() - t0

    timed(2)  # warm
    t1 = min(timed(1) for _ in range(3))
    tk = min(timed(iters) for _ in range(3))
    per_exec = (tk - t1) / (iters - 1)
    return per_exec, res0


def kernel(g_q, g_k, d_q, d_k, feat_q, feat_k, queue_g, queue_d):
    inputs = dict(g_q=g_q, g_k=g_k, d_q=d_q, d_k=d_k, feat_q=feat_q,
                  feat_k=feat_k, queue_g=queue_g, queue_d=queue_d)
    inputs = {k: np.asarray(v) for k, v in inputs.items()}
    out_g, out_d, _ = run_sharded(inputs, trace=False)
    target_g = np.zeros((BS,), dtype=np.int32)
    target_d = np.zeros((BS, S), dtype=np.int32)
    return out_g, target_g, out_d, target_d
